# revision 20
# baseline (speedup 1.0000x reference)
"""Multi-head causal attention (B=2, S=2048, D=1024, H=16) on 8 TRN2 NeuronCores.

Sharding: core c -> (head-group g = c//2 of 4 heads, batch half s = c%2).
Each core computes Q/K/V projections for its 4 heads over its batch element,
causal softmax attention, and a partial output projection (its 256 columns of
Wo). Host sums the 4 per-group partials for each batch element and adds bo.

Perf notes (vs the f32r baseline, 239us -> ~200us):
- Matmul operands are fp16/bf16 (fp32 PSUM accumulate); f32r weights
  disable FastWeightLoad and small-N f32r matmuls run at 1/4 rate. The
  exp path stays bf16: diagonal self-attention scores reach ~13 and
  exp(13) overflows fp16.
- Softmax reciprocal via the DVE reciprocal_approx_fast custom op (the
  stock Reciprocal runs the iterative-divide ALU at ~8 cyc/elem, 3.3us
  per [128,512] on HW). The custom DVE ops only work at base partition
  0, so the raw denominator rows are broadcast first (K=1 matmuls, as
  before) and the approx reciprocal runs on the broadcast result.
- A ~7us burst of dependency-free matmuls at kernel start warms the PE
  HAM clock gate (otherwise the first ~25us of projections run at
  1.2GHz instead of 2.4).
- Dedicated PSUM tags: scores->exp pipeline (s2, 4 banks) is decoupled
  from attn accumulators (att, 2 banks) and projection/out-proj chains
  (gen, 2 banks), so projection matmuls can fill the PE during the
  ACT(exp)-bound attention stretches; projections for q-tile jq+1 are
  emitted before out_proj(jq) for the same reason.
- Partial outputs written fp16 (halves the 8MB out DMA), summed on host.
- DMA triggers cost ~1us each on their issuing queue; inputs are merged
  into few large transfers (wqkv, bqkv) and spread across the SP and
  GpSimd queues so the fill is not trigger-serialized.
- Timed-loop builds use For_i(staggered_reset=True): the back edge skips
  the all-engine reset barrier (verified bit-identical across
  iterations), overlapping one iteration's tail with the next's fill.
"""

import contextlib
import sys

sys.path.insert(0, "/opt/trn_rl_repo")

import numpy as np

import concourse.bass as bass  # noqa: F401  (bass must import before bacc)
import concourse.mybir as mybir
from concourse import bacc
from concourse.bass_utils import run_bass_kernel_spmd
from concourse.tile import TileContext

F32 = mybir.dt.float32
F16 = mybir.dt.float16
F32R = mybir.dt.float32r
BF16 = mybir.dt.bfloat16
AF = mybir.ActivationFunctionType
ALU = mybir.AluOpType

B = 2
S = 2048            # sequence per batch element (= rows per core)
D = 1024            # embed dim
H = 16              # total heads
HD = 64             # head dim
DL = 256            # local dims per core (4 heads)
NI = D // 128       # 8 contraction tiles for projections
NQ = S // 512       # 4 query tiles of 512
NK = S // 128       # 16 key tiles of 128
SCALE = HD ** -0.5


def _build_nc(loop_iters=None, phases="full", warmup=True, stagger=True):
    nc = bacc.Bacc()

    xq_d = nc.declare_dram_parameter("xq_t", [128, NQ * NI * 512], BF16,
                                     isOutput=False)
    xk_d = nc.declare_dram_parameter("xk_t", [128, NQ * NI * 512], BF16,
                                     isOutput=False)
    xv_d = nc.declare_dram_parameter("xv_t", [128, NQ * NI * 512], BF16,
                                     isOutput=False)
    wqkv_d = nc.declare_dram_parameter("wqkv_t", [128, 3 * NI * DL], BF16,
                                       isOutput=False)
    wo_d = nc.declare_dram_parameter("wo_t", [128, 2 * D], F16, isOutput=False)
    bqkv_d = nc.declare_dram_parameter("bqkv", [128, 4 + 2 * DL], F32,
                                       isOutput=False)
    mk_d = nc.declare_dram_parameter("masks", [128, 4 * 1024], BF16, isOutput=False)
    on_d = nc.declare_dram_parameter("ones66", [66, 128], F32R, isOutput=False)
    # out[p, jn2*D + d] = partial_out[jn2*128 + p, d]; host un-permutes
    out_d = nc.declare_dram_parameter("out", [128, NK * D], F16, isOutput=True)

    with TileContext(nc) as tc:
        with tc.tile_pool(name="const", bufs=1) as cp, \
             tc.tile_pool(name="xpool", bufs=4) as xp, \
             tc.tile_pool(name="work", bufs=3) as wp, \
             tc.tile_pool(name="ps_s2", bufs=2, space="PSUM") as pp_s2, \
             tc.tile_pool(name="ps_att", bufs=2, space="PSUM") as pp_att, \
             tc.tile_pool(name="ps_gen", bufs=2, space="PSUM") as pp_gen:

            ET = mybir.EngineType
            loop_cm = (tc.For_i(0, loop_iters, 1,
                                hint_engines=(ET.PE, ET.DVE, ET.Activation,
                                              ET.SP, ET.Pool),
                                staggered_reset=stagger)
                       if loop_iters else contextlib.nullcontext())
            with loop_cm:
                # ---- persistent SBUF tensors ----
                wqkv_sb = cp.tile([128, 3 * NI * DL], BF16)
                wo_sb = cp.tile([128, 2 * D], F16)
                qt_sb = cp.tile([128, 2 * S], F16)   # Q^T: pair p cols [p*S:(p+1)*S]
                kt_sb = cp.tile([128, 2 * S], F16)
                at_sb = cp.tile([128, 2 * S], F16)   # attn out^T (normalized)
                va0 = cp.tile([128, NK * 65], BF16)   # head A of pair 0, +ones col 64
                va1 = cp.tile([128, NK * 65], BF16)
                vb0 = cp.tile([128, NK * 128], BF16)  # head B: col0=ones, 64:128=V
                vb1 = cp.tile([128, NK * 128], BF16)
                va = [va0, va1]
                vb = [vb0, vb1]
                mask_sb = cp.tile([128, 4 * 1024], BF16)
                ones_sb = cp.tile([66, 128], F32R)
                bqkv_sb = cp.tile([128, 4 + 2 * DL], F32)  # [bq/bk quad | bv x2]

                # warm the PE HAM clock gate during the DMA fill: a dense
                # burst of tiny matmuls on a memset tile (no DMA deps)
                if warmup:
                    wu = wp.tile([1, 640], BF16, tag="wu", name="wu")
                    nc.vector.memset(wu, 1.0)
                    ps_w = pp_att.tile([64, 512], F32, tag="att", bufs=2,
                                       name="warm")
                    for i in range(16):
                        nc.tensor.matmul(ps_w, wu[0:1, 0:64],
                                         wu[0:1, 128:640],
                                         start=(i == 0), stop=(i == 15))

                nc.sync.dma_start(out=wqkv_sb[:, 0:NI * DL],
                                  in_=wqkv_d[:, 0:NI * DL])

                # ---- phase 1: projections (Q/K in two 128-dim chains per
                # jn so each chain needs only one PSUM bank; V as before) ----
                def project(jn):
                    nsl = slice(jn * 512, (jn + 1) * 512)
                    xq_sl = xp.tile([128, NI * 512], BF16, tag="xq", bufs=2,
                                    name=f"xq_{jn}")
                    xk_sl = xp.tile([128, NI * 512], BF16, tag="xk", bufs=2,
                                    name=f"xk_{jn}")
                    ssl = slice(jn * NI * 512, (jn + 1) * NI * 512)
                    nc.gpsimd.dma_start(out=xq_sl, in_=xq_d[:, ssl])
                    nc.sync.dma_start(out=xk_sl, in_=xk_d[:, ssl])
                    if jn == 0:
                        nc.sync.dma_start(
                            out=wqkv_sb[:, NI * DL:3 * NI * DL],
                            in_=wqkv_d[:, NI * DL:3 * NI * DL])
                    xq_t = [xq_sl[:, ji * 512:(ji + 1) * 512] for ji in range(NI)]
                    xk_t = [xk_sl[:, ji * 512:(ji + 1) * 512] for ji in range(NI)]

                    if jn == 0:
                        # constants not needed until wave A eviction / wave B
                        # queue behind the first activation slabs
                        nc.gpsimd.dma_start(out=bqkv_sb, in_=bqkv_d[:])
                        for p in range(2):
                            nc.any.memset(va[p].rearrange(
                                "q (m c) -> q m c", c=65)[:, :, 64:65], 1.0)
                            nc.any.memset(vb[p].rearrange(
                                "q (m c) -> q m c", c=128)[:, :, 0:1], 1.0)

                    # wave A: Q/K projections as (t = head pair) chains
                    for t in range(2):
                        ps_q = pp_gen.tile([128, 512], F32, tag="gen", bufs=2,
                                           name=f"psq_{jn}_{t}")
                        ps_k = pp_gen.tile([128, 512], F32, tag="gen", bufs=2,
                                           name=f"psk_{jn}_{t}")
                        for ji in range(NI):
                            st, sp = ji == 0, ji == NI - 1
                            wsl = slice(ji * DL + t * 128,
                                        ji * DL + (t + 1) * 128)
                            nc.tensor.matmul(ps_q, wqkv_sb[:, wsl], xq_t[ji],
                                             start=st, stop=sp)
                            nc.tensor.matmul(
                                ps_k,
                                wqkv_sb[:, 2048 + wsl.start:2048 + wsl.stop],
                                xk_t[ji], start=st, stop=sp)
                        dst = slice(t * S + jn * 512, t * S + (jn + 1) * 512)
                        nc.vector.tensor_scalar(qt_sb[:, dst], ps_q,
                                                bqkv_sb[:, 2 * t:2 * t + 1],
                                                None, ALU.add)
                        nc.vector.tensor_scalar(kt_sb[:, dst], ps_k,
                                                bqkv_sb[:, 2 * t + 1:2 * t + 2],
                                                None, ALU.add)

                    # wave B: V projection (2 chains of 2 half-bank chains)
                    xv_sl = xp.tile([128, NI * 512], BF16, tag="xv", bufs=2,
                                    name=f"xv_{jn}")
                    nc.gpsimd.dma_start(out=xv_sl, in_=xv_d[:, ssl])
                    xv_t = [xv_sl[:, ji * 512:(ji + 1) * 512] for ji in range(NI)]
                    # two V chains share one PSUM bank: only the first
                    # matmul to touch the bank uses start=True (bank-wide
                    # has_written clear); the sibling chain's first matmul
                    # relies on per-element overwrite-when-bit-clear.
                    ps_v2 = [pp_gen.tile([128, 512], F32, tag="gen", bufs=2,
                                         name=f"psv2_{jn}_{w}")
                             for w in range(2)]
                    ps_v = [ps_v2[u // 2][:, (u % 2) * DL:(u % 2 + 1) * DL]
                            for u in range(4)]
                    for ji in range(NI):
                        sp = ji == NI - 1
                        for u in range(4):
                            nc.tensor.matmul(
                                ps_v[u],
                                xv_t[ji][:, u * 128:(u + 1) * 128],
                                wqkv_sb[:, 4096 + ji * DL:4096 + (ji + 1) * DL],
                                start=(ji == 0 and u % 2 == 0), stop=sp)
                    # biased eviction, two m-blocks per op ([q, 2, 64] views)
                    bvv = bqkv_sb[:, 4:4 + 2 * DL].rearrange(
                        "q (m c) -> q m c", c=DL)
                    for w in range(2):
                        m0 = jn * 4 + 2 * w
                        psv = ps_v2[w].rearrange("q (m x) -> q m x", x=DL)
                        for p in range(2):
                            ha = slice(p * 128, p * 128 + 64)
                            hb = slice(p * 128 + 64, p * 128 + 128)
                            nc.vector.tensor_tensor(
                                out=va[p].rearrange(
                                    "q (m c) -> q m c", c=65)[:, m0:m0 + 2, 0:64],
                                in0=psv[:, :, ha], in1=bvv[:, :, ha], op=ALU.add)
                            nc.vector.tensor_tensor(
                                out=vb[p].rearrange(
                                    "q (m c) -> q m c",
                                    c=128)[:, m0:m0 + 2, 64:128],
                                in0=psv[:, :, hb], in1=bvv[:, :, hb], op=ALU.add)

                # ---- phase 2: causal attention (head pairs packed on
                # partitions) ----
                def attention(p, jq):
                    qsl = slice(p * S + jq * 512, p * S + (jq + 1) * 512)
                    nk = 4 * jq + 4
                    ps_oa = pp_att.tile([65, 512], F32, tag="att", bufs=2,
                                        name=f"oa{p}_{jq}")
                    ps_ob = pp_att.tile([128, 512], F32, tag="att", bufs=2,
                                        name=f"ob{p}_{jq}")
                    for jk in range(nk):
                        d = jk - 4 * jq
                        c0 = 128 * d if d > 0 else 0  # first causally-valid col
                        ksl = slice(p * S + jk * 128, p * S + (jk + 1) * 128)
                        qsl_v = slice(p * S + jq * 512 + c0,
                                      p * S + (jq + 1) * 512)
                        ps_s2 = pp_s2.tile([128, 1024], F32, tag="s2", bufs=2,
                                           name=f"s2{p}_{jq}_{jk}")
                        nc.tensor.matmul(ps_s2[:, c0:512], kt_sb[0:64, ksl],
                                         qt_sb[0:64, qsl_v],
                                         start=True, stop=True,
                                         tile_position=(0, 0))
                        nc.tensor.matmul(ps_s2[:, 512 + c0:1024],
                                         kt_sb[64:128, ksl],
                                         qt_sb[64:128, qsl_v],
                                         start=True, stop=True,
                                         tile_position=(64, 0))
                        e2 = wp.tile([128, 1024], BF16, tag="e2", bufs=8,
                                     name=f"e2{p}_{jq}_{jk}")
                        s2v = ps_s2.rearrange("q (h n) -> q h n", n=512)[:, :, c0:]
                        e2v = e2.rearrange("q (h n) -> q h n", n=512)[:, :, c0:]
                        nc.scalar.activation(e2v, s2v, AF.Exp, scale=SCALE)
                        if d >= 0:  # diagonal block: the masked triangle
                            # spans exactly cols [c0, c0+128); beyond that
                            # q - k >= 128(d+1) - 127 > 128d, i.e. all valid
                            e2m = e2.rearrange(
                                "q (h n) -> q h n", n=512)[:, :, c0:c0 + 128]
                            mkm = mask_sb.rearrange(
                                "q (d h n) -> q d h n",
                                d=4, h=2)[:, d, :, c0:c0 + 128]
                            nc.vector.tensor_tensor(out=e2m, in0=e2m, in1=mkm,
                                                    op=ALU.mult)
                        st, sp = jk == 0, jk == nk - 1
                        nc.tensor.matmul(ps_oa[:, c0:512],
                                         va[p][:, jk * 65:(jk + 1) * 65],
                                         e2[:, c0:512], start=st, stop=sp)
                        nc.tensor.matmul(ps_ob[:, c0:512],
                                         vb[p][:, jk * 128:(jk + 1) * 128],
                                         e2[:, 512 + c0:1024], start=st, stop=sp)

                    # softmax denominators: copy the two raw rows to SBUF,
                    # broadcast across partitions via K=1 matmuls, then a
                    # fast approximate reciprocal (the custom DVE ops only
                    # work at base partition 0, so recip runs post-broadcast)
                    rsa = wp.tile([65, 512], F32R, tag="rsa", bufs=2,
                                  name=f"rsa{p}_{jq}")
                    rsb = wp.tile([1, 512], F32R, tag="rsb", bufs=2,
                                  name=f"rsb{p}_{jq}")
                    nc.vector.tensor_copy(rsa[64:65, :], ps_oa[64:65, :])
                    nc.vector.tensor_copy(rsb, ps_ob[0:1, :])
                    ps_ba = pp_gen.tile([128, 512], F32, tag="gen", bufs=2,
                                        name=f"ba{p}_{jq}")
                    ps_bb = pp_gen.tile([128, 512], F32, tag="gen", bufs=2,
                                        name=f"bb{p}_{jq}")
                    nc.tensor.matmul(ps_ba, ones_sb[64:65, :], rsa[64:65, :],
                                     start=True, stop=True)
                    nc.tensor.matmul(ps_bb, ones_sb[0:1, :], rsb[:],
                                     start=True, stop=True)
                    bca = wp.tile([128, 512], F32, tag="bca", bufs=2,
                                  name=f"bca{p}_{jq}")
                    bcb = wp.tile([128, 512], F32, tag="bcb", bufs=2,
                                  name=f"bcb{p}_{jq}")
                    nc.vector.reciprocal_approx_fast(out=bca, in_=ps_ba)
                    nc.vector.reciprocal_approx_fast(out=bcb, in_=ps_bb)
                    nc.vector.tensor_tensor(out=at_sb[0:64, qsl],
                                            in0=ps_oa[0:64, :],
                                            in1=bca[0:64, :],
                                            op=ALU.mult)
                    nc.vector.tensor_tensor(out=at_sb[64:128, qsl],
                                            in0=ps_ob[64:128, :],
                                            in1=bcb[64:128, :],
                                            op=ALU.mult)

                # ---- phase 3: partial out-projection, two 128-row tiles
                # per fp16 DMA ----
                def out_proj(j0, tail=False):
                    o_sb = wp.tile([128, 2048], F16, tag="osb", bufs=2,
                                   name=f"osb{j0}")
                    for jj in range(2):
                        jn2 = j0 + jj
                        for jo in range(2):
                            ps_o = pp_gen.tile([128, 512], F32, tag="gen",
                                               bufs=2, name=f"po{jn2}_{jo}")
                            for p in range(2):
                                nc.tensor.matmul(
                                    ps_o,
                                    at_sb[:, p * S + jn2 * 128:
                                          p * S + (jn2 + 1) * 128],
                                    wo_sb[:, p * D + jo * 512:
                                          p * D + (jo + 1) * 512],
                                    start=(p == 0), stop=(p == 1))
                            dst = o_sb[:, jj * 1024 + jo * 512:
                                       jj * 1024 + (jo + 1) * 512]
                            if tail:
                                nc.scalar.activation(dst, ps_o, AF.Copy)
                            else:
                                nc.vector.tensor_copy(dst, ps_o)
                        if tail:
                            nc.sync.dma_start(
                                out=out_d[:, jn2 * D:(jn2 + 1) * D],
                                in_=o_sb[:, jj * 1024:(jj + 1) * 1024])
                    if not tail:
                        nc.sync.dma_start(
                            out=out_d[:, j0 * D:(j0 + 2) * D], in_=o_sb)

                for jq in range(NQ):
                    # out-proj for the PREVIOUS q-tile is emitted before this
                    # tile's projection so its PSUM allocations sit ahead of
                    # the projection chains in the gen ring — its matmuls are
                    # ready the moment the prior epilogue lands and can fill
                    # the PE while ACT runs exp
                    if phases == "full" and jq > 0:
                        for j0 in range(4 * (jq - 1), 4 * jq, 2):
                            out_proj(j0)
                    project(jq)
                    if jq == 0:
                        # phase-2/3 constants load once phase 1 is underway
                        nc.gpsimd.dma_start(out=mask_sb, in_=mk_d[:])
                        nc.scalar.dma_start(out=wo_sb, in_=wo_d[:])
                        nc.gpsimd.dma_start(out=ones_sb, in_=on_d[:])
                    if phases == "p1":
                        continue
                    attention(0, jq)
                    attention(1, jq)
                if phases == "full":
                    for j0 in range(4 * (NQ - 1), 4 * NQ, 2):
                        out_proj(j0, tail=True)
                elif phases != "p1":
                    out_proj(0)

                if phases == "p1":  # dummy out write so `out` has a producer
                    dmy = wp.tile([128, 512], F16, tag="osb", name="dmy")
                    nc.vector.tensor_copy(dmy, qt_sb[:, 0:512])
                    nc.sync.dma_start(out=out_d[:, 0:512], in_=dmy)
    nc.finalize()
    return nc


_NC = {}


def _get_nc(loop_iters=None, phases="full", warmup=True, stagger=True):
    key = (loop_iters, phases, warmup, stagger)
    if key not in _NC:
        _NC[key] = _build_nc(loop_iters, phases, warmup, stagger)
    return _NC[key]


def _host_masks():
    kl = np.arange(128)[:, None]
    ql = np.arange(512)[None, :]
    blocks = []
    for d in range(4):
        m = (ql >= kl + 128 * d).astype(np.float32)
        blocks.append(np.concatenate([m, m], axis=1))  # A half | B half
    return np.concatenate(blocks, axis=1)


def build_in_maps(query, key_in, value, Wq, bq, Wk, bk, Wv, bv, Wo, bo):
    query = np.asarray(query, dtype=np.float32)
    key_in = np.asarray(key_in, dtype=np.float32)
    value = np.asarray(value, dtype=np.float32)
    Wq = np.asarray(Wq, dtype=np.float32)
    Wk = np.asarray(Wk, dtype=np.float32)
    Wv = np.asarray(Wv, dtype=np.float32)
    Wo = np.asarray(Wo, dtype=np.float32)
    bq = np.asarray(bq, dtype=np.float32)
    bk = np.asarray(bk, dtype=np.float32)
    bv = np.asarray(bv, dtype=np.float32)
    bo = np.asarray(bo, dtype=np.float32)

    import ml_dtypes
    bf16 = ml_dtypes.bfloat16
    masks = np.ascontiguousarray(_host_masks()).astype(bf16)
    def _pmajor_act(x):
        # [D, S] -> [128, jn*NI*512+a*512+n] with D = a*128+p, S = jn*512+n
        xt = np.ascontiguousarray(x.T)
        return np.ascontiguousarray(
            xt.reshape(NI, 128, NQ, 512).transpose(1, 2, 0, 3).reshape(
                128, NQ * NI * 512)).astype(bf16)

    def _pmajor_w(w):
        # [D, DL] -> [128, a*DL+m] with D = a*128+p
        return np.ascontiguousarray(
            w.reshape(NI, 128, DL).transpose(1, 0, 2).reshape(128, NI * DL))

    xq = [_pmajor_act(query[s]) for s in range(B)]
    xk = [_pmajor_act(key_in[s]) for s in range(B)]
    xv = [_pmajor_act(value[s]) for s in range(B)]

    in_maps = []
    for c in range(8):
        g, s = c // 2, c % 2
        dsl = slice(g * DL, (g + 1) * DL)
        bv_loc = np.broadcast_to(bv[None, dsl], (128, DL))
        in_maps.append({
            "xq_t": xq[s],
            "xk_t": xk[s],
            "xv_t": xv[s],
            "wqkv_t": np.ascontiguousarray(np.concatenate(
                [_pmajor_w(Wq[dsl, :].T), _pmajor_w(Wk[dsl, :].T),
                 _pmajor_w(Wv[dsl, :].T)], axis=1)).astype(bf16),
            "wo_t": np.ascontiguousarray(
                Wo[:, dsl].T.reshape(2, 128, D).transpose(1, 0, 2).reshape(
                    128, 2 * D)).astype(np.float16),
            "bqkv": np.ascontiguousarray(np.concatenate(
                [np.stack([bq[dsl][0:128], bk[dsl][0:128],
                           bq[dsl][128:256], bk[dsl][128:256]],
                          axis=1),
                 bv_loc, bv_loc], axis=1)),
            "masks": masks,
            "ones66": np.ones((66, 128), dtype=np.float32),
        })
    return in_maps


def kernel(query, key_in, value, Wq, bq, Wk, bk, Wv, bv, Wo, bo):
    bo = np.asarray(bo, dtype=np.float32)
    in_maps = build_in_maps(query, key_in, value, Wq, bq, Wk, bk, Wv, bv, Wo, bo)
    nc = _get_nc()
    res = run_bass_kernel_spmd(nc, in_maps, core_ids=list(range(8)))

    out = np.zeros((B, S, D), dtype=np.float32)
    for c in range(8):
        s = c % 2
        out[s] += res.results[c]["out"].astype(np.float32).reshape(
            128, NK, D).transpose(1, 0, 2).reshape(S, D)
    out += bo[None, None, :]
    return out


# revision 21
# speedup vs baseline: 1.0037x; 1.0037x over previous
"""Multi-head causal attention (B=2, S=2048, D=1024, H=16) on 8 TRN2 NeuronCores.

Sharding: core c -> (head-group g = c//2 of 4 heads, batch half s = c%2).
Each core computes Q/K/V projections for its 4 heads over its batch element,
causal softmax attention, and a partial output projection (its 256 columns of
Wo). Host sums the 4 per-group partials for each batch element and adds bo.

Perf notes (vs the f32r baseline, 239us -> ~200us):
- Matmul operands are fp16/bf16 (fp32 PSUM accumulate); f32r weights
  disable FastWeightLoad and small-N f32r matmuls run at 1/4 rate. The
  exp path stays bf16: diagonal self-attention scores reach ~13 and
  exp(13) overflows fp16.
- Softmax reciprocal via the DVE reciprocal_approx_fast custom op (the
  stock Reciprocal runs the iterative-divide ALU at ~8 cyc/elem, 3.3us
  per [128,512] on HW). The custom DVE ops only work at base partition
  0, so the raw denominator rows are broadcast first (K=1 matmuls, as
  before) and the approx reciprocal runs on the broadcast result.
- A ~7us burst of dependency-free matmuls at kernel start warms the PE
  HAM clock gate (otherwise the first ~25us of projections run at
  1.2GHz instead of 2.4).
- Dedicated PSUM tags: scores->exp pipeline (s2, 4 banks) is decoupled
  from attn accumulators (att, 2 banks) and projection/out-proj chains
  (gen, 2 banks), so projection matmuls can fill the PE during the
  ACT(exp)-bound attention stretches; projections for q-tile jq+1 are
  emitted before out_proj(jq) for the same reason.
- Partial outputs written fp16 (halves the 8MB out DMA), summed on host.
- DMA triggers cost ~1us each on their issuing queue; inputs are merged
  into few large transfers (wqkv, bqkv) and spread across the SP and
  GpSimd queues so the fill is not trigger-serialized.
- Timed-loop builds use For_i(staggered_reset=True): the back edge skips
  the all-engine reset barrier (verified bit-identical across
  iterations), overlapping one iteration's tail with the next's fill.
"""

import contextlib
import sys

sys.path.insert(0, "/opt/trn_rl_repo")

import numpy as np

import concourse.bass as bass  # noqa: F401  (bass must import before bacc)
import concourse.mybir as mybir
from concourse import bacc
from concourse.bass_utils import run_bass_kernel_spmd
from concourse.tile import TileContext

F32 = mybir.dt.float32
F16 = mybir.dt.float16
F32R = mybir.dt.float32r
BF16 = mybir.dt.bfloat16
AF = mybir.ActivationFunctionType
ALU = mybir.AluOpType

B = 2
S = 2048            # sequence per batch element (= rows per core)
D = 1024            # embed dim
H = 16              # total heads
HD = 64             # head dim
DL = 256            # local dims per core (4 heads)
NI = D // 128       # 8 contraction tiles for projections
NQ = S // 512       # 4 query tiles of 512
NK = S // 128       # 16 key tiles of 128
SCALE = HD ** -0.5


def _build_nc(loop_iters=None, phases="full", warmup=True, stagger=True):
    nc = bacc.Bacc()

    xq_d = nc.declare_dram_parameter("xq_t", [128, NQ * NI * 512], BF16,
                                     isOutput=False)
    xk_d = nc.declare_dram_parameter("xk_t", [128, NQ * NI * 512], BF16,
                                     isOutput=False)
    xv_d = nc.declare_dram_parameter("xv_t", [128, NQ * NI * 512], BF16,
                                     isOutput=False)
    wqkv_d = nc.declare_dram_parameter("wqkv_t", [128, 3 * NI * DL], BF16,
                                       isOutput=False)
    wo_d = nc.declare_dram_parameter("wo_t", [128, 2 * D], F16, isOutput=False)
    bqkv_d = nc.declare_dram_parameter("bqkv", [128, 4 + 2 * DL], F32,
                                       isOutput=False)
    mk_d = nc.declare_dram_parameter("masks", [128, 4 * 1024], BF16, isOutput=False)
    on_d = nc.declare_dram_parameter("ones66", [66, 128], F32R, isOutput=False)
    # out[p, jn2*D + d] = partial_out[jn2*128 + p, d]; host un-permutes
    out_d = nc.declare_dram_parameter("out", [128, NK * D], F16, isOutput=True)

    with TileContext(nc) as tc:
        with tc.tile_pool(name="const", bufs=1) as cp, \
             tc.tile_pool(name="xpool", bufs=4) as xp, \
             tc.tile_pool(name="work", bufs=3) as wp, \
             tc.tile_pool(name="ps_s2", bufs=2, space="PSUM") as pp_s2, \
             tc.tile_pool(name="ps_att", bufs=2, space="PSUM") as pp_att, \
             tc.tile_pool(name="ps_gen", bufs=2, space="PSUM") as pp_gen:

            ET = mybir.EngineType
            loop_cm = (tc.For_i(0, loop_iters, 1,
                                hint_engines=(ET.PE, ET.DVE, ET.Activation,
                                              ET.SP, ET.Pool),
                                staggered_reset=stagger)
                       if loop_iters else contextlib.nullcontext())
            with loop_cm:
                # ---- persistent SBUF tensors ----
                wqkv_sb = cp.tile([128, 3 * NI * DL], BF16)
                wo_sb = cp.tile([128, 2 * D], F16)
                qt_sb = cp.tile([128, 2 * S], F16)   # Q^T: pair p cols [p*S:(p+1)*S]
                kt_sb = cp.tile([128, 2 * S], F16)
                at_sb = cp.tile([128, 2 * S], F16)   # attn out^T (normalized)
                va0 = cp.tile([128, NK * 65], BF16)   # head A of pair 0, +ones col 64
                va1 = cp.tile([128, NK * 65], BF16)
                vb0 = cp.tile([128, NK * 128], BF16)  # head B: col0=ones, 64:128=V
                vb1 = cp.tile([128, NK * 128], BF16)
                va = [va0, va1]
                vb = [vb0, vb1]
                mask_sb = cp.tile([128, 4 * 1024], BF16)
                ones_sb = cp.tile([66, 128], F32R)
                bqkv_sb = cp.tile([128, 4 + 2 * DL], F32)  # [bq/bk quad | bv x2]

                # warm the PE HAM clock gate during the DMA fill: a dense
                # burst of tiny matmuls on a memset tile (no DMA deps)
                if warmup:
                    wu = wp.tile([1, 640], BF16, tag="wu", name="wu")
                    nc.vector.memset(wu, 1.0)
                    ps_w = pp_att.tile([64, 512], F32, tag="att", bufs=2,
                                       name="warm")
                    for i in range(16):
                        nc.tensor.matmul(ps_w, wu[0:1, 0:64],
                                         wu[0:1, 128:640],
                                         start=(i == 0), stop=(i == 15))

                nc.sync.dma_start(out=wqkv_sb[:, 0:NI * DL],
                                  in_=wqkv_d[:, 0:NI * DL])

                # ---- phase 1: projections (Q/K in two 128-dim chains per
                # jn so each chain needs only one PSUM bank; V as before) ----
                def project(jn):
                    nsl = slice(jn * 512, (jn + 1) * 512)
                    xq_sl = xp.tile([128, NI * 512], BF16, tag="xq", bufs=3,
                                    name=f"xq_{jn}")
                    xk_sl = xp.tile([128, NI * 512], BF16, tag="xk", bufs=3,
                                    name=f"xk_{jn}")
                    ssl = slice(jn * NI * 512, (jn + 1) * NI * 512)
                    nc.gpsimd.dma_start(out=xq_sl, in_=xq_d[:, ssl])
                    nc.sync.dma_start(out=xk_sl, in_=xk_d[:, ssl])
                    if jn == 0:
                        nc.sync.dma_start(
                            out=wqkv_sb[:, NI * DL:3 * NI * DL],
                            in_=wqkv_d[:, NI * DL:3 * NI * DL])
                    xq_t = [xq_sl[:, ji * 512:(ji + 1) * 512] for ji in range(NI)]
                    xk_t = [xk_sl[:, ji * 512:(ji + 1) * 512] for ji in range(NI)]

                    if jn == 0:
                        # constants not needed until wave A eviction / wave B
                        # queue behind the first activation slabs
                        nc.gpsimd.dma_start(out=bqkv_sb, in_=bqkv_d[:])
                        for p in range(2):
                            nc.any.memset(va[p].rearrange(
                                "q (m c) -> q m c", c=65)[:, :, 64:65], 1.0)
                            nc.any.memset(vb[p].rearrange(
                                "q (m c) -> q m c", c=128)[:, :, 0:1], 1.0)

                    # wave A: Q/K projections as (t = head pair) chains
                    for t in range(2):
                        ps_q = pp_gen.tile([128, 512], F32, tag="gen", bufs=2,
                                           name=f"psq_{jn}_{t}")
                        ps_k = pp_gen.tile([128, 512], F32, tag="gen", bufs=2,
                                           name=f"psk_{jn}_{t}")
                        for ji in range(NI):
                            st, sp = ji == 0, ji == NI - 1
                            wsl = slice(ji * DL + t * 128,
                                        ji * DL + (t + 1) * 128)
                            nc.tensor.matmul(ps_q, wqkv_sb[:, wsl], xq_t[ji],
                                             start=st, stop=sp)
                            nc.tensor.matmul(
                                ps_k,
                                wqkv_sb[:, 2048 + wsl.start:2048 + wsl.stop],
                                xk_t[ji], start=st, stop=sp)
                        dst = slice(t * S + jn * 512, t * S + (jn + 1) * 512)
                        nc.vector.tensor_scalar(qt_sb[:, dst], ps_q,
                                                bqkv_sb[:, 2 * t:2 * t + 1],
                                                None, ALU.add)
                        nc.vector.tensor_scalar(kt_sb[:, dst], ps_k,
                                                bqkv_sb[:, 2 * t + 1:2 * t + 2],
                                                None, ALU.add)

                    # wave B: V projection (2 chains of 2 half-bank chains)
                    xv_sl = xp.tile([128, NI * 512], BF16, tag="xv", bufs=3,
                                    name=f"xv_{jn}")
                    nc.gpsimd.dma_start(out=xv_sl, in_=xv_d[:, ssl])
                    xv_t = [xv_sl[:, ji * 512:(ji + 1) * 512] for ji in range(NI)]
                    # two V chains share one PSUM bank: only the first
                    # matmul to touch the bank uses start=True (bank-wide
                    # has_written clear); the sibling chain's first matmul
                    # relies on per-element overwrite-when-bit-clear.
                    ps_v2 = [pp_gen.tile([128, 512], F32, tag="gen", bufs=2,
                                         name=f"psv2_{jn}_{w}")
                             for w in range(2)]
                    ps_v = [ps_v2[u // 2][:, (u % 2) * DL:(u % 2 + 1) * DL]
                            for u in range(4)]
                    for ji in range(NI):
                        sp = ji == NI - 1
                        for u in range(4):
                            nc.tensor.matmul(
                                ps_v[u],
                                xv_t[ji][:, u * 128:(u + 1) * 128],
                                wqkv_sb[:, 4096 + ji * DL:4096 + (ji + 1) * DL],
                                start=(ji == 0 and u % 2 == 0), stop=sp)
                    # biased eviction, two m-blocks per op ([q, 2, 64] views)
                    bvv = bqkv_sb[:, 4:4 + 2 * DL].rearrange(
                        "q (m c) -> q m c", c=DL)
                    for w in range(2):
                        m0 = jn * 4 + 2 * w
                        psv = ps_v2[w].rearrange("q (m x) -> q m x", x=DL)
                        for p in range(2):
                            ha = slice(p * 128, p * 128 + 64)
                            hb = slice(p * 128 + 64, p * 128 + 128)
                            nc.vector.tensor_tensor(
                                out=va[p].rearrange(
                                    "q (m c) -> q m c", c=65)[:, m0:m0 + 2, 0:64],
                                in0=psv[:, :, ha], in1=bvv[:, :, ha], op=ALU.add)
                            nc.vector.tensor_tensor(
                                out=vb[p].rearrange(
                                    "q (m c) -> q m c",
                                    c=128)[:, m0:m0 + 2, 64:128],
                                in0=psv[:, :, hb], in1=bvv[:, :, hb], op=ALU.add)

                # ---- phase 2: causal attention (head pairs packed on
                # partitions) ----
                def attention(p, jq):
                    qsl = slice(p * S + jq * 512, p * S + (jq + 1) * 512)
                    nk = 4 * jq + 4
                    ps_oa = pp_att.tile([65, 512], F32, tag="att", bufs=2,
                                        name=f"oa{p}_{jq}")
                    ps_ob = pp_att.tile([128, 512], F32, tag="att", bufs=2,
                                        name=f"ob{p}_{jq}")
                    for jk in range(nk):
                        d = jk - 4 * jq
                        c0 = 128 * d if d > 0 else 0  # first causally-valid col
                        ksl = slice(p * S + jk * 128, p * S + (jk + 1) * 128)
                        qsl_v = slice(p * S + jq * 512 + c0,
                                      p * S + (jq + 1) * 512)
                        ps_s2 = pp_s2.tile([128, 1024], F32, tag="s2", bufs=2,
                                           name=f"s2{p}_{jq}_{jk}")
                        nc.tensor.matmul(ps_s2[:, c0:512], kt_sb[0:64, ksl],
                                         qt_sb[0:64, qsl_v],
                                         start=True, stop=True,
                                         tile_position=(0, 0))
                        nc.tensor.matmul(ps_s2[:, 512 + c0:1024],
                                         kt_sb[64:128, ksl],
                                         qt_sb[64:128, qsl_v],
                                         start=True, stop=True,
                                         tile_position=(64, 0))
                        e2 = wp.tile([128, 1024], BF16, tag="e2", bufs=8,
                                     name=f"e2{p}_{jq}_{jk}")
                        s2v = ps_s2.rearrange("q (h n) -> q h n", n=512)[:, :, c0:]
                        e2v = e2.rearrange("q (h n) -> q h n", n=512)[:, :, c0:]
                        nc.scalar.activation(e2v, s2v, AF.Exp, scale=SCALE)
                        if d >= 0:  # diagonal block: the masked triangle
                            # spans exactly cols [c0, c0+128); beyond that
                            # q - k >= 128(d+1) - 127 > 128d, i.e. all valid
                            e2m = e2.rearrange(
                                "q (h n) -> q h n", n=512)[:, :, c0:c0 + 128]
                            mkm = mask_sb.rearrange(
                                "q (d h n) -> q d h n",
                                d=4, h=2)[:, d, :, c0:c0 + 128]
                            nc.vector.tensor_tensor(out=e2m, in0=e2m, in1=mkm,
                                                    op=ALU.mult)
                        st, sp = jk == 0, jk == nk - 1
                        nc.tensor.matmul(ps_oa[:, c0:512],
                                         va[p][:, jk * 65:(jk + 1) * 65],
                                         e2[:, c0:512], start=st, stop=sp)
                        nc.tensor.matmul(ps_ob[:, c0:512],
                                         vb[p][:, jk * 128:(jk + 1) * 128],
                                         e2[:, 512 + c0:1024], start=st, stop=sp)

                    # softmax denominators: copy the two raw rows to SBUF,
                    # broadcast across partitions via K=1 matmuls, then a
                    # fast approximate reciprocal (the custom DVE ops only
                    # work at base partition 0, so recip runs post-broadcast)
                    rsa = wp.tile([65, 512], F32R, tag="rsa", bufs=2,
                                  name=f"rsa{p}_{jq}")
                    rsb = wp.tile([1, 512], F32R, tag="rsb", bufs=2,
                                  name=f"rsb{p}_{jq}")
                    nc.vector.tensor_copy(rsa[64:65, :], ps_oa[64:65, :])
                    nc.vector.tensor_copy(rsb, ps_ob[0:1, :])
                    ps_ba = pp_gen.tile([128, 512], F32, tag="gen", bufs=2,
                                        name=f"ba{p}_{jq}")
                    ps_bb = pp_gen.tile([128, 512], F32, tag="gen", bufs=2,
                                        name=f"bb{p}_{jq}")
                    nc.tensor.matmul(ps_ba, ones_sb[64:65, :], rsa[64:65, :],
                                     start=True, stop=True)
                    nc.tensor.matmul(ps_bb, ones_sb[0:1, :], rsb[:],
                                     start=True, stop=True)
                    bca = wp.tile([128, 512], F32, tag="bca", bufs=2,
                                  name=f"bca{p}_{jq}")
                    bcb = wp.tile([128, 512], F32, tag="bcb", bufs=2,
                                  name=f"bcb{p}_{jq}")
                    nc.vector.reciprocal_approx_fast(out=bca, in_=ps_ba)
                    nc.vector.reciprocal_approx_fast(out=bcb, in_=ps_bb)
                    nc.vector.tensor_tensor(out=at_sb[0:64, qsl],
                                            in0=ps_oa[0:64, :],
                                            in1=bca[0:64, :],
                                            op=ALU.mult)
                    nc.vector.tensor_tensor(out=at_sb[64:128, qsl],
                                            in0=ps_ob[64:128, :],
                                            in1=bcb[64:128, :],
                                            op=ALU.mult)

                # ---- phase 3: partial out-projection, two 128-row tiles
                # per fp16 DMA ----
                def out_proj(j0, tail=False):
                    o_sb = wp.tile([128, 2048], F16, tag="osb", bufs=2,
                                   name=f"osb{j0}")
                    for jj in range(2):
                        jn2 = j0 + jj
                        for jo in range(2):
                            ps_o = pp_gen.tile([128, 512], F32, tag="gen",
                                               bufs=2, name=f"po{jn2}_{jo}")
                            for p in range(2):
                                nc.tensor.matmul(
                                    ps_o,
                                    at_sb[:, p * S + jn2 * 128:
                                          p * S + (jn2 + 1) * 128],
                                    wo_sb[:, p * D + jo * 512:
                                          p * D + (jo + 1) * 512],
                                    start=(p == 0), stop=(p == 1))
                            dst = o_sb[:, jj * 1024 + jo * 512:
                                       jj * 1024 + (jo + 1) * 512]
                            if tail:
                                nc.scalar.activation(dst, ps_o, AF.Copy)
                            else:
                                nc.vector.tensor_copy(dst, ps_o)
                        if tail:
                            nc.sync.dma_start(
                                out=out_d[:, jn2 * D:(jn2 + 1) * D],
                                in_=o_sb[:, jj * 1024:(jj + 1) * 1024])
                    if not tail:
                        nc.sync.dma_start(
                            out=out_d[:, j0 * D:(j0 + 2) * D], in_=o_sb)

                for jq in range(NQ):
                    # out-proj for the PREVIOUS q-tile is emitted before this
                    # tile's projection so its PSUM allocations sit ahead of
                    # the projection chains in the gen ring — its matmuls are
                    # ready the moment the prior epilogue lands and can fill
                    # the PE while ACT runs exp
                    if phases == "full" and jq > 0:
                        for j0 in range(4 * (jq - 1), 4 * jq, 2):
                            out_proj(j0)
                    project(jq)
                    if jq == 0:
                        # phase-2/3 constants load once phase 1 is underway
                        nc.gpsimd.dma_start(out=mask_sb, in_=mk_d[:])
                        nc.scalar.dma_start(out=wo_sb, in_=wo_d[:])
                        nc.gpsimd.dma_start(out=ones_sb, in_=on_d[:])
                    if phases == "p1":
                        continue
                    attention(0, jq)
                    attention(1, jq)
                if phases == "full":
                    for j0 in range(4 * (NQ - 1), 4 * NQ, 2):
                        out_proj(j0, tail=True)
                elif phases != "p1":
                    out_proj(0)

                if phases == "p1":  # dummy out write so `out` has a producer
                    dmy = wp.tile([128, 512], F16, tag="osb", name="dmy")
                    nc.vector.tensor_copy(dmy, qt_sb[:, 0:512])
                    nc.sync.dma_start(out=out_d[:, 0:512], in_=dmy)
    nc.finalize()
    return nc


_NC = {}


def _get_nc(loop_iters=None, phases="full", warmup=True, stagger=True):
    key = (loop_iters, phases, warmup, stagger)
    if key not in _NC:
        _NC[key] = _build_nc(loop_iters, phases, warmup, stagger)
    return _NC[key]


def _host_masks():
    kl = np.arange(128)[:, None]
    ql = np.arange(512)[None, :]
    blocks = []
    for d in range(4):
        m = (ql >= kl + 128 * d).astype(np.float32)
        blocks.append(np.concatenate([m, m], axis=1))  # A half | B half
    return np.concatenate(blocks, axis=1)


def build_in_maps(query, key_in, value, Wq, bq, Wk, bk, Wv, bv, Wo, bo):
    query = np.asarray(query, dtype=np.float32)
    key_in = np.asarray(key_in, dtype=np.float32)
    value = np.asarray(value, dtype=np.float32)
    Wq = np.asarray(Wq, dtype=np.float32)
    Wk = np.asarray(Wk, dtype=np.float32)
    Wv = np.asarray(Wv, dtype=np.float32)
    Wo = np.asarray(Wo, dtype=np.float32)
    bq = np.asarray(bq, dtype=np.float32)
    bk = np.asarray(bk, dtype=np.float32)
    bv = np.asarray(bv, dtype=np.float32)
    bo = np.asarray(bo, dtype=np.float32)

    import ml_dtypes
    bf16 = ml_dtypes.bfloat16
    masks = np.ascontiguousarray(_host_masks()).astype(bf16)
    def _pmajor_act(x):
        # [D, S] -> [128, jn*NI*512+a*512+n] with D = a*128+p, S = jn*512+n
        xt = np.ascontiguousarray(x.T)
        return np.ascontiguousarray(
            xt.reshape(NI, 128, NQ, 512).transpose(1, 2, 0, 3).reshape(
                128, NQ * NI * 512)).astype(bf16)

    def _pmajor_w(w):
        # [D, DL] -> [128, a*DL+m] with D = a*128+p
        return np.ascontiguousarray(
            w.reshape(NI, 128, DL).transpose(1, 0, 2).reshape(128, NI * DL))

    xq = [_pmajor_act(query[s]) for s in range(B)]
    xk = [_pmajor_act(key_in[s]) for s in range(B)]
    xv = [_pmajor_act(value[s]) for s in range(B)]

    in_maps = []
    for c in range(8):
        g, s = c // 2, c % 2
        dsl = slice(g * DL, (g + 1) * DL)
        bv_loc = np.broadcast_to(bv[None, dsl], (128, DL))
        in_maps.append({
            "xq_t": xq[s],
            "xk_t": xk[s],
            "xv_t": xv[s],
            "wqkv_t": np.ascontiguousarray(np.concatenate(
                [_pmajor_w(Wq[dsl, :].T), _pmajor_w(Wk[dsl, :].T),
                 _pmajor_w(Wv[dsl, :].T)], axis=1)).astype(bf16),
            "wo_t": np.ascontiguousarray(
                Wo[:, dsl].T.reshape(2, 128, D).transpose(1, 0, 2).reshape(
                    128, 2 * D)).astype(np.float16),
            "bqkv": np.ascontiguousarray(np.concatenate(
                [np.stack([bq[dsl][0:128], bk[dsl][0:128],
                           bq[dsl][128:256], bk[dsl][128:256]],
                          axis=1),
                 bv_loc, bv_loc], axis=1)),
            "masks": masks,
            "ones66": np.ones((66, 128), dtype=np.float32),
        })
    return in_maps


def kernel(query, key_in, value, Wq, bq, Wk, bk, Wv, bv, Wo, bo):
    bo = np.asarray(bo, dtype=np.float32)
    in_maps = build_in_maps(query, key_in, value, Wq, bq, Wk, bk, Wv, bv, Wo, bo)
    nc = _get_nc()
    res = run_bass_kernel_spmd(nc, in_maps, core_ids=list(range(8)))

    out = np.zeros((B, S, D), dtype=np.float32)
    for c in range(8):
        s = c % 2
        out[s] += res.results[c]["out"].astype(np.float32).reshape(
            128, NK, D).transpose(1, 0, 2).reshape(S, D)
    out += bo[None, None, :]
    return out
